# revision 2
# baseline (speedup 1.0000x reference)
"""DKVMN forward kernel v2 for 8 Trainium2 NeuronCores (Bass/Tile).

Changes vs v1 (54.3ms baseline):
 - All three big tensor_tensor multiplies use bf16 operands with packed
   innermost (time) dims via small transposed staging tiles -> DVE 2x_1p.
 - e|a tables combined into one 512B-row table -> one gather, not two.
 - Scan + reduce split across DVE (range A) and GpSimd (range B) with
   separate D/U tiles per range.
 - Head fully fused into the chunk loop: reads are PE-transposed to
   k-on-partition layout, f = tanh(fW1@read + fW2@k + fb) accumulated in
   PSUM via PE, pred = sigmoid(pW.f + pb) via PE contraction. No DRAM
   round-trips, no KFtab/G1 stages.
"""
import sys
import numpy as np
import ml_dtypes

sys.path.insert(0, '/opt/trn_rl_repo')

import concourse.bass as bass          # noqa: E402
import concourse.bacc as bacc          # noqa: E402
import concourse.mybir as mybir        # noqa: E402
from concourse.tile import TileContext # noqa: E402
from concourse.bass_utils import run_bass_kernel_spmd  # noqa: E402

F32 = mybir.dt.float32
BF16 = mybir.dt.bfloat16
I16 = mybir.dt.int16
ALU = mybir.AluOpType
ACTF = mybir.ActivationFunctionType
BF = ml_dtypes.bfloat16

NUM_ITEM = 2000
DK = 256          # key dim
DV = 128          # memory slots (v)
B, T = 256, 512
NC = 8
BL = B // NC      # 32 local batches
KSUB = 4          # k quarters on partitions
K2 = DK // KSUB   # 64
P = BL * KSUB     # 128 partitions: p = b*4 + ksub
SEG = K2 * DV     # 8192 cells per partition (k2, v)
C = 4             # scan chunk length
SLOT = C + 1
NCH = T // C      # 128 chunks
NIT = 2048        # padded item count
NX = 4096         # padded x count
TOK = BL * T      # tokens per core

KA = 40           # k2 split: rmult-A on DVE, rmult-B on Pool; V/U on Pool
POOL_TT = False   # bisect: Pool tensor_tensor on/off
NA = KA * DV      # cells in range A
NB = SEG - NA

_cache = {}


def _wrap16(vals):
    n = len(vals)
    assert n % 16 == 0
    a = np.zeros((16, n // 16), np.int16)
    for i in range(n):
        a[i % 16, i // 16] = vals[i]
    return np.tile(a, (8, 1))


def build_program():
    nc = bacc.Bacc(None, target_bir_lowering=False, debug=False)

    # ---- external inputs ----
    kT = nc.dram_tensor("kT", [DK, NIT], BF16, kind="ExternalInput")       # k_emb^T padded
    vT = nc.dram_tensor("vT", [DK, NX], BF16, kind="ExternalInput")        # v_emb^T padded
    MkT = nc.dram_tensor("MkT", [DK, DV], BF16, kind="ExternalInput")      # Mk^T
    eaWT = nc.dram_tensor("eaWT", [DK, 2 * DK], BF16, kind="ExternalInput")
    onesf = nc.dram_tensor("onesf", [1, 128], F32, kind="ExternalInput")
    eab = nc.dram_tensor("eab", [1, 2 * DK], F32, kind="ExternalInput")
    kR = nc.dram_tensor("kR", [NIT, DK], BF16, kind="ExternalInput")       # k_emb rows (for transpose-gather)
    fw1L = nc.dram_tensor("fw1L", [K2, KSUB * 2 * 128], BF16, kind="ExternalInput")
    fw2L = nc.dram_tensor("fw2L", [128, 2 * 2 * 128], BF16, kind="ExternalInput")
    fbL = nc.dram_tensor("fbL", [1, DK], F32, kind="ExternalInput")
    pL = nc.dram_tensor("pL", [128, 2], BF16, kind="ExternalInput")
    pb1 = nc.dram_tensor("pb1", [1, 1], F32, kind="ExternalInput")
    ident = nc.dram_tensor("ident", [128, 128], F32, kind="ExternalInput")
    m0sh = nc.dram_tensor("m0sh", [P, SEG], BF16, kind="ExternalInput")    # M0 shifted by one cell
    m0c0 = nc.dram_tensor("m0c0", [P, 1], BF16, kind="ExternalInput")      # M0 of cell 0
    cidx = nc.dram_tensor("cidx", [P, NCH, 72], I16, kind="ExternalInput") # w(32)|ea(32)|k(8)

    pred = nc.dram_tensor("pred", [1, NCH * 128], F32, kind="ExternalOutput")

    # ---- DRAM scratch ----
    Wtab = nc.dram_tensor("Wtab", [NIT, DV], F32)                  # softmax rows
    EAQTab = nc.dram_tensor("EAQTab", [NX * KSUB, 2 * K2], F32)    # [e_q | a_q] 512B rows

    with TileContext(nc) as tc:
        # ================= stage 1: tables =================
        with (
            tc.tile_pool(name="wpool", bufs=1) as wp,
            tc.tile_pool(name="tpool", bufs=1) as tp,
            tc.tile_pool(name="pspool", bufs=2, space="PSUM") as pp,
        ):
            kT_s = [wp.tile([128, NIT], BF16, tag=f"kt{i}", name=f"kt{i}") for i in range(2)]
            vT_s = [wp.tile([128, NX], BF16, tag=f"vt{i}", name=f"vt{i}") for i in range(2)]
            MkT_s = [wp.tile([128, DV], BF16, tag=f"mk{i}", name=f"mk{i}") for i in range(2)]
            eaWT_s = [wp.tile([128, 2 * DK], BF16, tag=f"ea{i}", name=f"eaw{i}") for i in range(2)]
            onesf_s = wp.tile([1, 128], F32, tag="onf")
            eab_s = wp.tile([1, 2 * DK], F32, tag="eb")
            for i in range(2):
                nc.sync.dma_start(kT_s[i][:], kT[128 * i:128 * (i + 1), :])
                nc.sync.dma_start(vT_s[i][:], vT[128 * i:128 * (i + 1), :])
                nc.sync.dma_start(MkT_s[i][:], MkT[128 * i:128 * (i + 1), :])
                nc.sync.dma_start(eaWT_s[i][:], eaWT[128 * i:128 * (i + 1), :])
            nc.sync.dma_start(onesf_s[:], onesf[:])
            nc.sync.dma_start(eab_s[:], eab[:])

            # --- Wtab: softmax(k_emb @ Mk^T) ---
            wexp = tp.tile([128, 16, DV], F32, tag="wexp")
            for it in range(16):
                ps = pp.tile([128, DV], F32, tag="ps_w")
                sl = slice(128 * it, 128 * (it + 1))
                nc.tensor.matmul(out=ps[:], lhsT=kT_s[0][:, sl], rhs=MkT_s[0][:],
                                 start=True, stop=False)
                nc.tensor.matmul(out=ps[:], lhsT=kT_s[1][:, sl], rhs=MkT_s[1][:],
                                 start=False, stop=True)
                nc.scalar.activation(out=wexp[:, it, :], in_=ps[:], func=ACTF.Exp)
            zs = tp.tile([128, 16], F32, tag="zs")
            nc.vector.tensor_reduce(out=zs[:], in_=wexp[:], axis=mybir.AxisListType.X,
                                    op=ALU.add)
            zr = tp.tile([128, 16], F32, tag="zr")
            nc.vector.reciprocal(out=zr[:], in_=zs[:])
            nc.vector.tensor_tensor(
                out=wexp[:], in0=wexp[:],
                in1=zr[:].unsqueeze(2).to_broadcast([128, 16, DV]), op=ALU.mult)
            nc.sync.dma_start(
                Wtab[:].rearrange("(it p) v -> p it v", p=128), wexp[:])

            # --- EAQTab: rows (x*4+q) = [sig(e)_q | tanh(a)_q] ---
            ea = tp.tile([128, 32, 2 * DK], F32, tag="ea")
            for it in range(32):
                ps = pp.tile([128, 2 * DK], F32, tag="ps_ea")
                sl = slice(128 * it, 128 * (it + 1))
                nc.tensor.matmul(out=ps[:], lhsT=vT_s[0][:, sl], rhs=eaWT_s[0][:],
                                 start=True, stop=False)
                nc.tensor.matmul(out=ps[:], lhsT=vT_s[1][:, sl], rhs=eaWT_s[1][:],
                                 start=False, stop=False)
                nc.tensor.matmul(out=ps[:], lhsT=onesf_s[:], rhs=eab_s[:],
                                 start=False, stop=True)
                nc.scalar.activation(out=ea[:, it, 0:DK], in_=ps[:, 0:DK], func=ACTF.Sigmoid)
                nc.scalar.activation(out=ea[:, it, DK:2 * DK], in_=ps[:, DK:2 * DK],
                                     func=ACTF.Tanh)
            eaq = EAQTab[:].rearrange("(it p q) c -> p it q c", p=128, q=KSUB)
            for q in range(KSUB):
                nc.sync.dma_start(
                    eaq[:, :, q, 0:K2], ea[:, :, q * K2:(q + 1) * K2])
                nc.sync.dma_start(
                    eaq[:, :, q, K2:2 * K2], ea[:, :, DK + q * K2:DK + (q + 1) * K2])

        # ================= stage 2: fused scan + head =================
        with (
            tc.tile_pool(name="hw", bufs=1) as hw,
            tc.tile_pool(name="scst", bufs=1) as st,
            tc.tile_pool(name="scg", bufs=2) as sg,
            tc.tile_pool(name="hps", bufs=2, space="PSUM") as hps,
        ):
            # persistent weights
            fw1a_s = hw.tile([KA, KSUB, 2, 128], BF16, tag="fw1a")
            fw1b_s = hw.tile([K2 - KA, KSUB, 2, 128], BF16, tag="fw1b")
            fw2_s = hw.tile([128, 2, 2, 128], BF16, tag="fw2")
            fb_s = hw.tile([1, DK], F32, tag="fb")
            p_s = hw.tile([128, 2], BF16, tag="pl")
            pb_s = hw.tile([1, 1], F32, tag="pb")
            id_s = hw.tile([128, 128], F32, tag="id")
            ones_s = hw.tile([1, 128], F32, tag="on2")
            nc.sync.dma_start(fw1a_s[:].rearrange("a b c d -> a (b c d)"), fw1L[0:KA, :])
            nc.sync.dma_start(fw1b_s[:].rearrange("a b c d -> a (b c d)"), fw1L[KA:K2, :])
            nc.sync.dma_start(fw2_s[:].rearrange("a b c d -> a (b c d)"), fw2L[:])
            nc.sync.dma_start(fb_s[:], fbL[:])
            nc.sync.dma_start(p_s[:], pL[:])
            nc.sync.dma_start(pb_s[:], pb1[:])
            nc.sync.dma_start(id_s[:], ident[:])
            nc.sync.dma_start(ones_s[:], onesf[:])

            # state tiles per range
            D_a = st.tile([P, NA * SLOT], BF16, tag="Da")
            D_b = st.tile([P, NB * SLOT], BF16, tag="Db")
            U_a = st.tile([P, 1 + NA * SLOT], BF16, tag="Ua")
            U_b = st.tile([P, 1 + NB * SLOT], BF16, tag="Ub")

            # U init: pad col = M0(first cell of range); slot-C of cell s = M0(cell s+1)
            nc.sync.dma_start(U_a[:, 0:1], m0c0[:])
            nc.sync.dma_start(U_b[:, 0:1], m0sh[:, NA - 1:NA])
            # stage m0sh through D data area (D fully rewritten below: memset
            # zeroes the boundary slots AFTER these staged values are consumed)
            m0st_a = D_a[:, 0:NA]
            m0st_b = D_b[:, 0:NB]
            nc.sync.dma_start(m0st_a, m0sh[:, 0:NA])
            nc.sync.dma_start(m0st_b, m0sh[:, NA:SEG])
            nc.vector.tensor_copy(
                out=U_a[:, 1:].rearrange("p (s j) -> p s j", j=SLOT)[:, :, C:].squeeze(2),
                in_=m0st_a)
            nc.vector.tensor_copy(
                out=U_b[:, 1:].rearrange("p (s j) -> p s j", j=SLOT)[:, :, C:].squeeze(2),
                in_=m0st_b)
            # D boundary slots = 0 (after staging reads)
            nc.vector.memset(D_a[:].rearrange("p (s j) -> p s j", j=SLOT)[:, :, C:], 0.0)
            nc.vector.memset(D_b[:].rearrange("p (s j) -> p s j", j=SLOT)[:, :, C:], 0.0)

            d5a = D_a[:].rearrange("p (k v j) -> p k v j", k=KA, j=SLOT)
            d5b = D_b[:].rearrange("p (k v j) -> p k v j", k=K2 - KA, j=SLOT)
            ua_data = U_a[:, 1:].rearrange("p (k v j) -> p k v j", k=KA, j=SLOT)
            ub_data = U_b[:, 1:].rearrange("p (k v j) -> p k v j", k=K2 - KA, j=SLOT)

            for ch in range(NCH):
                ix = sg.tile([P, 72], I16, tag="ix")
                nc.sync.dma_start(ix[:], cidx[:, ch, :])
                w_g = sg.tile([P, C, DV], F32, tag="wg")
                ea_g = sg.tile([P, C, 2 * K2], F32, tag="eag")
                kTg = sg.tile([128, 2, 128], BF16, tag="ktg")
                nc.gpsimd.dma_gather(w_g[:], Wtab[:], ix[:, 0:32], C * P, C * P, DV)
                nc.gpsimd.dma_gather(ea_g[:], EAQTab[:], ix[:, 32:64], C * P, C * P,
                                     2 * K2)
                nc.gpsimd.dma_gather(kTg[:], kR[:], ix[:, 64:72], 128, 128, DK,
                                     transpose=True)

                # transposed bf16 staging (Act): [P, t, x] -> [P, x, t]
                wT = sg.tile([P, DV, C], BF16, tag="wt")
                eT = sg.tile([P, K2, C], BF16, tag="et")
                aT = sg.tile([P, K2, C], BF16, tag="at")
                nc.scalar.copy(out=wT[:], in_=w_g[:].transpose([0, 2, 1]))
                nc.scalar.copy(out=eT[:], in_=ea_g[:, :, 0:K2].transpose([0, 2, 1]))
                nc.scalar.copy(out=aT[:], in_=ea_g[:, :, K2:2 * K2].transpose([0, 2, 1]))

                # broadcast APs (outer dims may broadcast; innermost t packed)
                def w_ap(nk):
                    return wT[:].unsqueeze(1).to_broadcast([P, nk, DV, C])

                def ea_ap(x, nk, lo):
                    return x[:, lo:lo + nk, :].unsqueeze(2).to_broadcast([P, nk, DV, C])

                for (d5, u_data, U, D, nk, lo) in (
                    (d5a, ua_data, U_a, D_a, KA, 0),
                    (d5b, ub_data, U_b, D_b, K2 - KA, KA),
                ):
                    # V = w*e -> D (Pool); D = 1 - V (DVE 4x)
                    eng_tt = nc.gpsimd if POOL_TT else nc.vector
                    eng_tt.tensor_tensor(out=d5[:, :, :, 0:C], in0=w_ap(nk),
                                         in1=ea_ap(eT, nk, lo), op=ALU.mult)
                    nc.vector.tensor_scalar(out=d5[:, :, :, 0:C], in0=d5[:, :, :, 0:C],
                                            scalar1=-1.0, scalar2=1.0,
                                            op0=ALU.mult, op1=ALU.add)
                    # U = w*a (Pool)
                    eng_tt.tensor_tensor(out=u_data[:, :, :, 0:C], in0=w_ap(nk),
                                         in1=ea_ap(aT, nk, lo), op=ALU.mult)
                    # scan (DVE only; compiler rejects scan on Pool)
                    nc.vector.tensor_tensor_scan(
                        out=U[:, 1:], data0=D[:], data1=U[:, 1:],
                        initial=U[:, 0:1], op0=ALU.mult, op1=ALU.add)

                # reads: P = M_(t-1) * w -> D; reduce over v
                rT_a = sg.tile([P, KA, C], F32, tag="rta")
                rT_b = sg.tile([P, K2 - KA, C], F32, tag="rtb")
                mprev_a = U_a[:, 0:NA * SLOT].rearrange(
                    "p (k v j) -> p k v j", k=KA, j=SLOT)[:, :, :, 0:C]
                mprev_b = U_b[:, 0:NB * SLOT].rearrange(
                    "p (k v j) -> p k v j", k=K2 - KA, j=SLOT)[:, :, :, 0:C]
                (nc.gpsimd if POOL_TT else nc.vector).tensor_tensor(
                    out=d5b[:, :, :, 0:C], in0=mprev_b,
                    in1=w_ap(K2 - KA), op=ALU.mult)
                nc.vector.tensor_tensor(out=d5a[:, :, :, 0:C], in0=mprev_a,
                                        in1=w_ap(KA), op=ALU.mult)
                nc.vector.tensor_reduce(
                    out=rT_a[:], in_=d5a[:, :, :, 0:C].transpose([0, 1, 3, 2]),
                    axis=mybir.AxisListType.X, op=ALU.add)
                nc.vector.tensor_reduce(
                    out=rT_b[:], in_=d5b[:, :, :, 0:C].transpose([0, 1, 3, 2]),
                    axis=mybir.AxisListType.X, op=ALU.add)

                if ch + 1 < NCH:
                    # carries (Act): pad <- end-state(cell0); slot-C(s) <- end-state(s+1)
                    for (U, n) in ((U_a, NA), (U_b, NB)):
                        nc.scalar.copy(out=U[:, 0:1], in_=U[:, C:C + 1])
                        nc.scalar.copy(
                            out=U[:, 1:].rearrange("p (s j) -> p s j", j=SLOT)[:, 0:n - 1, C:],
                            in_=U[:, 1:].rearrange("p (s j) -> p s j", j=SLOT)[:, 1:n, C - 1:C])

                # ---- fused head ----
                # transpose reads to k-on-partition: readT_ps[k2, t*128 + (b,ks)]
                rps_a = hps.tile([KA, C, 128], F32, tag="rpsa")
                rps_b = hps.tile([K2 - KA, C, 128], F32, tag="rpsb")
                for t in range(C):
                    nc.tensor.transpose(out=rps_a[:, t, :], in_=rT_a[:, :, t],
                                        identity=id_s[:])
                    nc.tensor.transpose(out=rps_b[:, t, :], in_=rT_b[:, :, t],
                                        identity=id_s[:])
                rsb_a = sg.tile([KA, KSUB, BL, C], BF16, tag="rsba")
                rsb_b = sg.tile([K2 - KA, KSUB, BL, C], BF16, tag="rsbb")
                # psum col t*128 + b*4 + ks -> rsb[k2, ks, b, t]
                nc.scalar.copy(out=rsb_a[:], in_=rps_a[:].rearrange(
                    "k t (b q) -> k q b t", q=KSUB))
                nc.scalar.copy(out=rsb_b[:], in_=rps_b[:].rearrange(
                    "k t (b q) -> k q b t", q=KSUB))

                # fpre[dk, tok] = fW1@read + fW2@kemb + fb  (tok = b*4+t)
                fpre = hps.tile([128, 2, 128], F32, tag="fpre")
                for m in range(2):
                    for q in range(KSUB):
                        nc.tensor.matmul(
                            out=fpre[:, m, :], lhsT=fw1a_s[:, q, m, :],
                            rhs=rsb_a[:, q, :, :], start=(q == 0), stop=False)
                        nc.tensor.matmul(
                            out=fpre[:, m, :], lhsT=fw1b_s[:, q, m, :],
                            rhs=rsb_b[:, q, :, :], start=False, stop=False)
                    for c2 in range(2):
                        nc.tensor.matmul(
                            out=fpre[:, m, :], lhsT=fw2_s[:, c2, m, :],
                            rhs=kTg[:, c2, :], start=False, stop=False)
                    nc.tensor.matmul(
                        out=fpre[:, m, :], lhsT=fb_s[:, 128 * m:128 * (m + 1)],
                        rhs=ones_s[:], start=False, stop=True)
                fT = sg.tile([128, 2, 128], BF16, tag="ft")
                nc.scalar.activation(out=fT[:], in_=fpre[:], func=ACTF.Tanh)

                # pred = sigmoid(sum_dk p*f + pb)
                pp2 = hps.tile([1, 128], F32, tag="pp2")
                nc.tensor.matmul(out=pp2[:], lhsT=p_s[:, 0:1], rhs=fT[:, 0, :],
                                 start=True, stop=False)
                nc.tensor.matmul(out=pp2[:], lhsT=p_s[:, 1:2], rhs=fT[:, 1, :],
                                 start=False, stop=True)
                pr_sb = sg.tile([1, 128], F32, tag="prsb")
                nc.scalar.activation(out=pr_sb[:], in_=pp2[:], func=ACTF.Sigmoid,
                                     bias=pb_s[:])
                nc.sync.dma_start(pred[0:1, ch * 128:(ch + 1) * 128], pr_sb[:])

    nc.finalize()
    return nc


def _host_prep(item_seq, correct_seq, k_emb, v_emb, Mk, Mv0, e_W, e_b, a_W, a_b,
               f_W, f_b, p_W, p_b):
    pad_k = np.zeros((NIT, DK), np.float32)
    pad_k[:NUM_ITEM] = k_emb
    pad_v = np.zeros((NX, DK), np.float32)
    pad_v[:2 * NUM_ITEM] = v_emb
    fW1 = f_W[:, :DK]
    fW2 = f_W[:, DK:]
    # fw1L[k2, (q, m, j)] = fW1[m*128+j, q*64+k2]
    fw1L = np.ascontiguousarray(
        fW1.reshape(2, 128, KSUB, K2).transpose(3, 2, 0, 1).reshape(K2, -1))
    # fw2L[p, (c, m, j)] = fW2[m*128+j, c*128+p]
    fw2L = np.ascontiguousarray(
        fW2.reshape(2, 128, 2, 128).transpose(3, 2, 0, 1).reshape(128, -1))
    pLv = np.ascontiguousarray(p_W.reshape(2, 128).T)  # [p, c]
    shared = {
        "kT": np.ascontiguousarray(pad_k.T).astype(BF),
        "vT": np.ascontiguousarray(pad_v.T).astype(BF),
        "MkT": np.ascontiguousarray(Mk.T).astype(BF),
        "eaWT": np.ascontiguousarray(np.concatenate([e_W.T, a_W.T], axis=1)).astype(BF),
        "onesf": np.ones((1, 128), np.float32),
        "eab": np.concatenate([e_b, a_b])[None, :].astype(np.float32),
        "kR": pad_k.astype(BF),
        "fw1L": fw1L.astype(BF),
        "fw2L": fw2L.astype(BF),
        "fbL": f_b[None, :].astype(np.float32),
        "pL": pLv.astype(BF),
        "pb1": np.array([[float(p_b[0])]], np.float32),
        "ident": np.eye(128, dtype=np.float32),
    }
    ks = np.arange(P) % KSUB
    k2i, vi = np.meshgrid(np.arange(K2), np.arange(DV), indexing="ij")
    m0_cell = Mv0.T[(ks[:, None, None] * K2 + k2i[None]), vi[None]]  # [P, K2, DV]
    m0_flat = m0_cell.reshape(P, SEG).astype(np.float32)
    m0sh = np.zeros((P, SEG), np.float32)
    m0sh[:, :SEG - 1] = m0_flat[:, 1:]
    shared["m0sh"] = m0sh.astype(BF)
    shared["m0c0"] = m0_flat[:, 0:1].astype(BF)
    return shared


def _core_idx(item_c, x_c):
    """Per-core gather index tensor. item_c/x_c: [BL, T] int arrays."""
    bl = np.arange(P) // KSUB
    ks = np.arange(P) % KSUB
    cidx = np.zeros((P, NCH, 72), np.int16)
    for ch in range(NCH):
        tt = ch * C + np.arange(C)
        witem = item_c[bl[None, :], tt[:, None]].reshape(-1)              # [C*P], i=t*128+p
        eaidx = (x_c[bl[None, :], tt[:, None]] * KSUB + ks[None, :]).reshape(-1)
        # kT transpose-gather: col i = b*4+t -> item[b, ch*4+t]
        kit = item_c[:, tt].reshape(-1)                                   # [BL*C], i=b*4+t
        cidx[:, ch, 0:32] = _wrap16(witem.astype(np.int64))
        cidx[:, ch, 32:64] = _wrap16(eaidx.astype(np.int64))
        cidx[:, ch, 64:72] = _wrap16(kit.astype(np.int64))
    return {"cidx": cidx}


def unpack_pred(pred_row):
    """pred [1, NCH*128] -> [BL, T]; col = ch*128 + b*4 + t."""
    pr = np.asarray(pred_row).reshape(NCH, BL, C)    # [ch, b, t]
    return pr.transpose(1, 0, 2).reshape(BL, T)


def kernel(**inputs):
    inputs = {k: np.asarray(v) for k, v in inputs.items()}
    item = inputs["item_seq"].astype(np.int64)
    corr = inputs["correct_seq"].astype(np.int64)
    x = item + NUM_ITEM * corr

    if "nc" not in _cache:
        _cache["nc"] = build_program()
    nc = _cache["nc"]

    shared = _host_prep(
        item, corr,
        inputs["k_emb"].astype(np.float32), inputs["v_emb"].astype(np.float32),
        inputs["Mk"].astype(np.float32), inputs["Mv0"].astype(np.float32),
        inputs["e_W"].astype(np.float32), inputs["e_b"].astype(np.float32),
        inputs["a_W"].astype(np.float32), inputs["a_b"].astype(np.float32),
        inputs["f_W"].astype(np.float32), inputs["f_b"].astype(np.float32),
        inputs["p_W"].astype(np.float32), inputs["p_b"].astype(np.float32))

    in_maps = []
    for c in range(NC):
        sl = slice(c * BL, (c + 1) * BL)
        m = dict(shared)
        m.update(_core_idx(item[sl], x[sl]))
        in_maps.append(m)

    res = run_bass_kernel_spmd(nc, in_maps, core_ids=list(range(NC)))

    out = np.zeros((B, T), np.float32)
    for c in range(NC):
        out[c * BL:(c + 1) * BL] = unpack_pred(res.results[c]["pred"])
    return out


# revision 3
# speedup vs baseline: 1.0002x; 1.0002x over previous
"""DKVMN forward kernel v2 for 8 Trainium2 NeuronCores (Bass/Tile).

Changes vs v1 (54.3ms baseline):
 - All three big tensor_tensor multiplies use bf16 operands with packed
   innermost (time) dims via small transposed staging tiles -> DVE 2x_1p.
 - e|a tables combined into one 512B-row table -> one gather, not two.
 - Scan + reduce split across DVE (range A) and GpSimd (range B) with
   separate D/U tiles per range.
 - Head fully fused into the chunk loop: reads are PE-transposed to
   k-on-partition layout, f = tanh(fW1@read + fW2@k + fb) accumulated in
   PSUM via PE, pred = sigmoid(pW.f + pb) via PE contraction. No DRAM
   round-trips, no KFtab/G1 stages.
"""
import sys
import numpy as np
import ml_dtypes

sys.path.insert(0, '/opt/trn_rl_repo')

import concourse.bass as bass          # noqa: E402
import concourse.bacc as bacc          # noqa: E402
import concourse.mybir as mybir        # noqa: E402
from concourse.tile import TileContext # noqa: E402
from concourse.bass_utils import run_bass_kernel_spmd  # noqa: E402

F32 = mybir.dt.float32
BF16 = mybir.dt.bfloat16
I16 = mybir.dt.int16
ALU = mybir.AluOpType
ACTF = mybir.ActivationFunctionType
BF = ml_dtypes.bfloat16

NUM_ITEM = 2000
DK = 256          # key dim
DV = 128          # memory slots (v)
B, T = 256, 512
NC = 8
BL = B // NC      # 32 local batches
KSUB = 4          # k quarters on partitions
K2 = DK // KSUB   # 64
P = BL * KSUB     # 128 partitions: p = b*4 + ksub
SEG = K2 * DV     # 8192 cells per partition (k2, v)
C = 4             # scan chunk length
SLOT = C + 1
NCH = T // C      # 128 chunks
NIT = 2048        # padded item count
NX = 4096         # padded x count
TOK = BL * T      # tokens per core

KA = 40           # k2 split: rmult-A on DVE, rmult-B on Pool; V/U on Pool
POOL_TT = False   # bisect: Pool tensor_tensor on/off
NA = KA * DV      # cells in range A
NB = SEG - NA

_cache = {}


def _wrap16(vals):
    n = len(vals)
    assert n % 16 == 0
    a = np.zeros((16, n // 16), np.int16)
    for i in range(n):
        a[i % 16, i // 16] = vals[i]
    return np.tile(a, (8, 1))


def build_program():
    nc = bacc.Bacc(None, target_bir_lowering=False, debug=False)

    # ---- external inputs ----
    kT = nc.dram_tensor("kT", [DK, NIT], BF16, kind="ExternalInput")       # k_emb^T padded
    vT = nc.dram_tensor("vT", [DK, NX], BF16, kind="ExternalInput")        # v_emb^T padded
    MkT = nc.dram_tensor("MkT", [DK, DV], BF16, kind="ExternalInput")      # Mk^T
    eaWT = nc.dram_tensor("eaWT", [DK, 2 * DK], BF16, kind="ExternalInput")
    onesf = nc.dram_tensor("onesf", [1, 128], F32, kind="ExternalInput")
    eab = nc.dram_tensor("eab", [1, 2 * DK], F32, kind="ExternalInput")
    kR = nc.dram_tensor("kR", [NIT, DK], BF16, kind="ExternalInput")       # k_emb rows (for transpose-gather)
    fw1L = nc.dram_tensor("fw1L", [K2, KSUB * 2 * 128], BF16, kind="ExternalInput")
    fw2L = nc.dram_tensor("fw2L", [128, 2 * 2 * 128], BF16, kind="ExternalInput")
    fbL = nc.dram_tensor("fbL", [1, DK], F32, kind="ExternalInput")
    pL = nc.dram_tensor("pL", [128, 2], BF16, kind="ExternalInput")
    pb1 = nc.dram_tensor("pb1", [1, 1], F32, kind="ExternalInput")
    ident = nc.dram_tensor("ident", [128, 128], F32, kind="ExternalInput")
    m0sh = nc.dram_tensor("m0sh", [P, SEG], BF16, kind="ExternalInput")    # M0 shifted by one cell
    m0c0 = nc.dram_tensor("m0c0", [P, 1], BF16, kind="ExternalInput")      # M0 of cell 0
    cidx = nc.dram_tensor("cidx", [P, NCH, 72], I16, kind="ExternalInput") # w(32)|ea(32)|k(8)

    pred = nc.dram_tensor("pred", [1, NCH * 128], F32, kind="ExternalOutput")

    # ---- DRAM scratch ----
    Wtab = nc.dram_tensor("Wtab", [NIT, DV], F32)                  # softmax rows
    EAQTab = nc.dram_tensor("EAQTab", [NX * KSUB, 2 * K2], F32)    # [e_q | a_q] 512B rows

    with TileContext(nc) as tc:
        # ================= stage 1: tables =================
        with (
            tc.tile_pool(name="wpool", bufs=1) as wp,
            tc.tile_pool(name="tpool", bufs=1) as tp,
            tc.tile_pool(name="pspool", bufs=2, space="PSUM") as pp,
        ):
            kT_s = [wp.tile([128, NIT], BF16, tag=f"kt{i}", name=f"kt{i}") for i in range(2)]
            vT_s = [wp.tile([128, NX], BF16, tag=f"vt{i}", name=f"vt{i}") for i in range(2)]
            MkT_s = [wp.tile([128, DV], BF16, tag=f"mk{i}", name=f"mk{i}") for i in range(2)]
            eaWT_s = [wp.tile([128, 2 * DK], BF16, tag=f"ea{i}", name=f"eaw{i}") for i in range(2)]
            onesf_s = wp.tile([1, 128], F32, tag="onf")
            eab_s = wp.tile([1, 2 * DK], F32, tag="eb")
            for i in range(2):
                nc.sync.dma_start(kT_s[i][:], kT[128 * i:128 * (i + 1), :])
                nc.sync.dma_start(vT_s[i][:], vT[128 * i:128 * (i + 1), :])
                nc.sync.dma_start(MkT_s[i][:], MkT[128 * i:128 * (i + 1), :])
                nc.sync.dma_start(eaWT_s[i][:], eaWT[128 * i:128 * (i + 1), :])
            nc.sync.dma_start(onesf_s[:], onesf[:])
            nc.sync.dma_start(eab_s[:], eab[:])

            # --- Wtab: softmax(k_emb @ Mk^T) ---
            wexp = tp.tile([128, 16, DV], F32, tag="wexp")
            for it in range(16):
                ps = pp.tile([128, DV], F32, tag="ps_w")
                sl = slice(128 * it, 128 * (it + 1))
                nc.tensor.matmul(out=ps[:], lhsT=kT_s[0][:, sl], rhs=MkT_s[0][:],
                                 start=True, stop=False)
                nc.tensor.matmul(out=ps[:], lhsT=kT_s[1][:, sl], rhs=MkT_s[1][:],
                                 start=False, stop=True)
                nc.scalar.activation(out=wexp[:, it, :], in_=ps[:], func=ACTF.Exp)
            zs = tp.tile([128, 16], F32, tag="zs")
            nc.vector.tensor_reduce(out=zs[:], in_=wexp[:], axis=mybir.AxisListType.X,
                                    op=ALU.add)
            zr = tp.tile([128, 16], F32, tag="zr")
            nc.vector.reciprocal(out=zr[:], in_=zs[:])
            nc.vector.tensor_tensor(
                out=wexp[:], in0=wexp[:],
                in1=zr[:].unsqueeze(2).to_broadcast([128, 16, DV]), op=ALU.mult)
            nc.sync.dma_start(
                Wtab[:].rearrange("(it p) v -> p it v", p=128), wexp[:])

            # --- EAQTab: rows (x*4+q) = [sig(e)_q | tanh(a)_q] ---
            ea = tp.tile([128, 32, 2 * DK], F32, tag="ea")
            for it in range(32):
                ps = pp.tile([128, 2 * DK], F32, tag="ps_ea")
                sl = slice(128 * it, 128 * (it + 1))
                nc.tensor.matmul(out=ps[:], lhsT=vT_s[0][:, sl], rhs=eaWT_s[0][:],
                                 start=True, stop=False)
                nc.tensor.matmul(out=ps[:], lhsT=vT_s[1][:, sl], rhs=eaWT_s[1][:],
                                 start=False, stop=False)
                nc.tensor.matmul(out=ps[:], lhsT=onesf_s[:], rhs=eab_s[:],
                                 start=False, stop=True)
                nc.scalar.activation(out=ea[:, it, 0:DK], in_=ps[:, 0:DK], func=ACTF.Sigmoid)
                nc.scalar.activation(out=ea[:, it, DK:2 * DK], in_=ps[:, DK:2 * DK],
                                     func=ACTF.Tanh)
            eaq = EAQTab[:].rearrange("(it p q) c -> p it q c", p=128, q=KSUB)
            for q in range(KSUB):
                nc.sync.dma_start(
                    eaq[:, :, q, 0:K2], ea[:, :, q * K2:(q + 1) * K2])
                nc.sync.dma_start(
                    eaq[:, :, q, K2:2 * K2], ea[:, :, DK + q * K2:DK + (q + 1) * K2])

        # ================= stage 2: fused scan + head =================
        with (
            tc.tile_pool(name="hw", bufs=1) as hw,
            tc.tile_pool(name="scst", bufs=1) as st,
            tc.tile_pool(name="scg", bufs=2) as sg,
            tc.tile_pool(name="hps", bufs=2, space="PSUM") as hps,
        ):
            # persistent weights
            fw1a_s = hw.tile([KA, KSUB, 2, 128], BF16, tag="fw1a")
            fw1b_s = hw.tile([K2 - KA, KSUB, 2, 128], BF16, tag="fw1b")
            fw2_s = hw.tile([128, 2, 2, 128], BF16, tag="fw2")
            fb_s = hw.tile([1, DK], F32, tag="fb")
            p_s = hw.tile([128, 2], BF16, tag="pl")
            pb_s = hw.tile([1, 1], F32, tag="pb")
            id_s = hw.tile([128, 128], F32, tag="id")
            ones_s = hw.tile([1, 128], F32, tag="on2")
            nc.sync.dma_start(fw1a_s[:].rearrange("a b c d -> a (b c d)"), fw1L[0:KA, :])
            nc.sync.dma_start(fw1b_s[:].rearrange("a b c d -> a (b c d)"), fw1L[KA:K2, :])
            nc.sync.dma_start(fw2_s[:].rearrange("a b c d -> a (b c d)"), fw2L[:])
            nc.sync.dma_start(fb_s[:], fbL[:])
            nc.sync.dma_start(p_s[:], pL[:])
            nc.sync.dma_start(pb_s[:], pb1[:])
            nc.sync.dma_start(id_s[:], ident[:])
            nc.sync.dma_start(ones_s[:], onesf[:])

            # state tiles per range
            D_a = st.tile([P, NA * SLOT], BF16, tag="Da")
            D_b = st.tile([P, NB * SLOT], BF16, tag="Db")
            U_a = st.tile([P, 1 + NA * SLOT], BF16, tag="Ua")
            U_b = st.tile([P, 1 + NB * SLOT], BF16, tag="Ub")

            # U init: pad col = M0(first cell of range); slot-C of cell s = M0(cell s+1)
            nc.sync.dma_start(U_a[:, 0:1], m0c0[:])
            nc.sync.dma_start(U_b[:, 0:1], m0sh[:, NA - 1:NA])
            # stage m0sh through D data area (D fully rewritten below: memset
            # zeroes the boundary slots AFTER these staged values are consumed)
            m0st_a = D_a[:, 0:NA]
            m0st_b = D_b[:, 0:NB]
            nc.sync.dma_start(m0st_a, m0sh[:, 0:NA])
            nc.sync.dma_start(m0st_b, m0sh[:, NA:SEG])
            nc.vector.tensor_copy(
                out=U_a[:, 1:].rearrange("p (s j) -> p s j", j=SLOT)[:, :, C:].squeeze(2),
                in_=m0st_a)
            nc.vector.tensor_copy(
                out=U_b[:, 1:].rearrange("p (s j) -> p s j", j=SLOT)[:, :, C:].squeeze(2),
                in_=m0st_b)
            # D boundary slots = 0 (after staging reads)
            nc.vector.memset(D_a[:].rearrange("p (s j) -> p s j", j=SLOT)[:, :, C:], 0.0)
            nc.vector.memset(D_b[:].rearrange("p (s j) -> p s j", j=SLOT)[:, :, C:], 0.0)

            d5a = D_a[:].rearrange("p (k v j) -> p k v j", k=KA, j=SLOT)
            d5b = D_b[:].rearrange("p (k v j) -> p k v j", k=K2 - KA, j=SLOT)
            ua_data = U_a[:, 1:].rearrange("p (k v j) -> p k v j", k=KA, j=SLOT)
            ub_data = U_b[:, 1:].rearrange("p (k v j) -> p k v j", k=K2 - KA, j=SLOT)

            for ch in range(NCH):
                ix = sg.tile([P, 72], I16, tag="ix")
                nc.sync.dma_start(ix[:], cidx[:, ch, :])
                w_g = sg.tile([P, C, DV], F32, tag="wg")
                ea_g = sg.tile([P, C, 2 * K2], F32, tag="eag")
                kTg = sg.tile([128, 2, 128], BF16, tag="ktg")
                nc.gpsimd.dma_gather(w_g[:], Wtab[:], ix[:, 0:32], C * P, C * P, DV)
                nc.gpsimd.dma_gather(ea_g[:], EAQTab[:], ix[:, 32:64], C * P, C * P,
                                     2 * K2)
                nc.gpsimd.dma_gather(kTg[:], kR[:], ix[:, 64:72], 128, 128, DK,
                                     transpose=True)

                # transposed bf16 staging (Act): [P, t, x] -> [P, x, t]
                wT = sg.tile([P, DV, C], BF16, tag="wt")
                eT = sg.tile([P, K2, C], BF16, tag="et")
                aT = sg.tile([P, K2, C], BF16, tag="at")
                nc.scalar.copy(out=wT[:], in_=w_g[:].transpose([0, 2, 1]))
                nc.scalar.copy(out=eT[:], in_=ea_g[:, :, 0:K2].transpose([0, 2, 1]))
                nc.scalar.copy(out=aT[:], in_=ea_g[:, :, K2:2 * K2].transpose([0, 2, 1]))

                # broadcast APs (outer dims may broadcast; innermost t packed)
                def w_ap(nk):
                    return wT[:].unsqueeze(1).to_broadcast([P, nk, DV, C])

                def ea_ap(x, nk, lo):
                    return x[:, lo:lo + nk, :].unsqueeze(2).to_broadcast([P, nk, DV, C])

                for (d5, u_data, U, D, nk, lo) in (
                    (d5a, ua_data, U_a, D_a, KA, 0),
                    (d5b, ub_data, U_b, D_b, K2 - KA, KA),
                ):
                    # V = w*e -> D (Pool); D = 1 - V (DVE 4x)
                    eng_tt = nc.gpsimd if POOL_TT else nc.vector
                    eng_tt.tensor_tensor(out=d5[:, :, :, 0:C], in0=w_ap(nk),
                                         in1=ea_ap(eT, nk, lo), op=ALU.mult)
                    nc.scalar.activation(out=d5[:, :, :, 0:C], in_=d5[:, :, :, 0:C],
                                         func=ACTF.Copy, bias=1.0, scale=-1.0)
                    # U = w*a (Pool)
                    eng_tt.tensor_tensor(out=u_data[:, :, :, 0:C], in0=w_ap(nk),
                                         in1=ea_ap(aT, nk, lo), op=ALU.mult)
                    # scan (DVE only; compiler rejects scan on Pool)
                    nc.vector.tensor_tensor_scan(
                        out=U[:, 1:], data0=D[:], data1=U[:, 1:],
                        initial=U[:, 0:1], op0=ALU.mult, op1=ALU.add)

                # reads: P = M_(t-1) * w -> D; reduce over v
                rT_a = sg.tile([P, KA, C], F32, tag="rta")
                rT_b = sg.tile([P, K2 - KA, C], F32, tag="rtb")
                mprev_a = U_a[:, 0:NA * SLOT].rearrange(
                    "p (k v j) -> p k v j", k=KA, j=SLOT)[:, :, :, 0:C]
                mprev_b = U_b[:, 0:NB * SLOT].rearrange(
                    "p (k v j) -> p k v j", k=K2 - KA, j=SLOT)[:, :, :, 0:C]
                (nc.gpsimd if POOL_TT else nc.vector).tensor_tensor(
                    out=d5b[:, :, :, 0:C], in0=mprev_b,
                    in1=w_ap(K2 - KA), op=ALU.mult)
                nc.vector.tensor_tensor(out=d5a[:, :, :, 0:C], in0=mprev_a,
                                        in1=w_ap(KA), op=ALU.mult)
                H = DV // 2
                nc.vector.tensor_tensor(
                    out=d5a[:, :, 0:H, 0:C], in0=d5a[:, :, 0:H, 0:C],
                    in1=d5a[:, :, H:DV, 0:C], op=ALU.add)
                nc.vector.tensor_tensor(
                    out=d5b[:, :, 0:H, 0:C], in0=d5b[:, :, 0:H, 0:C],
                    in1=d5b[:, :, H:DV, 0:C], op=ALU.add)
                nc.vector.tensor_reduce(
                    out=rT_a[:], in_=d5a[:, :, 0:H, 0:C].transpose([0, 1, 3, 2]),
                    axis=mybir.AxisListType.X, op=ALU.add)
                nc.vector.tensor_reduce(
                    out=rT_b[:], in_=d5b[:, :, 0:H, 0:C].transpose([0, 1, 3, 2]),
                    axis=mybir.AxisListType.X, op=ALU.add)

                if ch + 1 < NCH:
                    # carries (Act): pad <- end-state(cell0); slot-C(s) <- end-state(s+1)
                    for (U, n) in ((U_a, NA), (U_b, NB)):
                        nc.scalar.copy(out=U[:, 0:1], in_=U[:, C:C + 1])
                        nc.scalar.copy(
                            out=U[:, 1:].rearrange("p (s j) -> p s j", j=SLOT)[:, 0:n - 1, C:],
                            in_=U[:, 1:].rearrange("p (s j) -> p s j", j=SLOT)[:, 1:n, C - 1:C])

                # ---- fused head ----
                # transpose reads to k-on-partition: readT_ps[k2, t*128 + (b,ks)]
                rps_a = hps.tile([KA, C, 128], F32, tag="rpsa")
                rps_b = hps.tile([K2 - KA, C, 128], F32, tag="rpsb")
                for t in range(C):
                    nc.tensor.transpose(out=rps_a[:, t, :], in_=rT_a[:, :, t],
                                        identity=id_s[:])
                    nc.tensor.transpose(out=rps_b[:, t, :], in_=rT_b[:, :, t],
                                        identity=id_s[:])
                rsb_a = sg.tile([KA, KSUB, BL, C], BF16, tag="rsba")
                rsb_b = sg.tile([K2 - KA, KSUB, BL, C], BF16, tag="rsbb")
                # psum col t*128 + b*4 + ks -> rsb[k2, ks, b, t]
                nc.scalar.copy(out=rsb_a[:], in_=rps_a[:].rearrange(
                    "k t (b q) -> k q b t", q=KSUB))
                nc.scalar.copy(out=rsb_b[:], in_=rps_b[:].rearrange(
                    "k t (b q) -> k q b t", q=KSUB))

                # fpre[dk, tok] = fW1@read + fW2@kemb + fb  (tok = b*4+t)
                fpre = hps.tile([128, 2, 128], F32, tag="fpre")
                for m in range(2):
                    for q in range(KSUB):
                        nc.tensor.matmul(
                            out=fpre[:, m, :], lhsT=fw1a_s[:, q, m, :],
                            rhs=rsb_a[:, q, :, :], start=(q == 0), stop=False)
                        nc.tensor.matmul(
                            out=fpre[:, m, :], lhsT=fw1b_s[:, q, m, :],
                            rhs=rsb_b[:, q, :, :], start=False, stop=False)
                    for c2 in range(2):
                        nc.tensor.matmul(
                            out=fpre[:, m, :], lhsT=fw2_s[:, c2, m, :],
                            rhs=kTg[:, c2, :], start=False, stop=False)
                    nc.tensor.matmul(
                        out=fpre[:, m, :], lhsT=fb_s[:, 128 * m:128 * (m + 1)],
                        rhs=ones_s[:], start=False, stop=True)
                fT = sg.tile([128, 2, 128], BF16, tag="ft")
                nc.scalar.activation(out=fT[:], in_=fpre[:], func=ACTF.Tanh)

                # pred = sigmoid(sum_dk p*f + pb)
                pp2 = hps.tile([1, 128], F32, tag="pp2")
                nc.tensor.matmul(out=pp2[:], lhsT=p_s[:, 0:1], rhs=fT[:, 0, :],
                                 start=True, stop=False)
                nc.tensor.matmul(out=pp2[:], lhsT=p_s[:, 1:2], rhs=fT[:, 1, :],
                                 start=False, stop=True)
                pr_sb = sg.tile([1, 128], F32, tag="prsb")
                nc.scalar.activation(out=pr_sb[:], in_=pp2[:], func=ACTF.Sigmoid,
                                     bias=pb_s[:])
                nc.sync.dma_start(pred[0:1, ch * 128:(ch + 1) * 128], pr_sb[:])

    nc.finalize()
    return nc


def _host_prep(item_seq, correct_seq, k_emb, v_emb, Mk, Mv0, e_W, e_b, a_W, a_b,
               f_W, f_b, p_W, p_b):
    pad_k = np.zeros((NIT, DK), np.float32)
    pad_k[:NUM_ITEM] = k_emb
    pad_v = np.zeros((NX, DK), np.float32)
    pad_v[:2 * NUM_ITEM] = v_emb
    fW1 = f_W[:, :DK]
    fW2 = f_W[:, DK:]
    # fw1L[k2, (q, m, j)] = fW1[m*128+j, q*64+k2]
    fw1L = np.ascontiguousarray(
        fW1.reshape(2, 128, KSUB, K2).transpose(3, 2, 0, 1).reshape(K2, -1))
    # fw2L[p, (c, m, j)] = fW2[m*128+j, c*128+p]
    fw2L = np.ascontiguousarray(
        fW2.reshape(2, 128, 2, 128).transpose(3, 2, 0, 1).reshape(128, -1))
    pLv = np.ascontiguousarray(p_W.reshape(2, 128).T)  # [p, c]
    shared = {
        "kT": np.ascontiguousarray(pad_k.T).astype(BF),
        "vT": np.ascontiguousarray(pad_v.T).astype(BF),
        "MkT": np.ascontiguousarray(Mk.T).astype(BF),
        "eaWT": np.ascontiguousarray(np.concatenate([e_W.T, a_W.T], axis=1)).astype(BF),
        "onesf": np.ones((1, 128), np.float32),
        "eab": np.concatenate([e_b, a_b])[None, :].astype(np.float32),
        "kR": pad_k.astype(BF),
        "fw1L": fw1L.astype(BF),
        "fw2L": fw2L.astype(BF),
        "fbL": f_b[None, :].astype(np.float32),
        "pL": pLv.astype(BF),
        "pb1": np.array([[float(p_b[0])]], np.float32),
        "ident": np.eye(128, dtype=np.float32),
    }
    ks = np.arange(P) % KSUB
    k2i, vi = np.meshgrid(np.arange(K2), np.arange(DV), indexing="ij")
    m0_cell = Mv0.T[(ks[:, None, None] * K2 + k2i[None]), vi[None]]  # [P, K2, DV]
    m0_flat = m0_cell.reshape(P, SEG).astype(np.float32)
    m0sh = np.zeros((P, SEG), np.float32)
    m0sh[:, :SEG - 1] = m0_flat[:, 1:]
    shared["m0sh"] = m0sh.astype(BF)
    shared["m0c0"] = m0_flat[:, 0:1].astype(BF)
    return shared


def _core_idx(item_c, x_c):
    """Per-core gather index tensor. item_c/x_c: [BL, T] int arrays."""
    bl = np.arange(P) // KSUB
    ks = np.arange(P) % KSUB
    cidx = np.zeros((P, NCH, 72), np.int16)
    for ch in range(NCH):
        tt = ch * C + np.arange(C)
        witem = item_c[bl[None, :], tt[:, None]].reshape(-1)              # [C*P], i=t*128+p
        eaidx = (x_c[bl[None, :], tt[:, None]] * KSUB + ks[None, :]).reshape(-1)
        # kT transpose-gather: col i = b*4+t -> item[b, ch*4+t]
        kit = item_c[:, tt].reshape(-1)                                   # [BL*C], i=b*4+t
        cidx[:, ch, 0:32] = _wrap16(witem.astype(np.int64))
        cidx[:, ch, 32:64] = _wrap16(eaidx.astype(np.int64))
        cidx[:, ch, 64:72] = _wrap16(kit.astype(np.int64))
    return {"cidx": cidx}


def unpack_pred(pred_row):
    """pred [1, NCH*128] -> [BL, T]; col = ch*128 + b*4 + t."""
    pr = np.asarray(pred_row).reshape(NCH, BL, C)    # [ch, b, t]
    return pr.transpose(1, 0, 2).reshape(BL, T)


def kernel(**inputs):
    inputs = {k: np.asarray(v) for k, v in inputs.items()}
    item = inputs["item_seq"].astype(np.int64)
    corr = inputs["correct_seq"].astype(np.int64)
    x = item + NUM_ITEM * corr

    if "nc" not in _cache:
        _cache["nc"] = build_program()
    nc = _cache["nc"]

    shared = _host_prep(
        item, corr,
        inputs["k_emb"].astype(np.float32), inputs["v_emb"].astype(np.float32),
        inputs["Mk"].astype(np.float32), inputs["Mv0"].astype(np.float32),
        inputs["e_W"].astype(np.float32), inputs["e_b"].astype(np.float32),
        inputs["a_W"].astype(np.float32), inputs["a_b"].astype(np.float32),
        inputs["f_W"].astype(np.float32), inputs["f_b"].astype(np.float32),
        inputs["p_W"].astype(np.float32), inputs["p_b"].astype(np.float32))

    in_maps = []
    for c in range(NC):
        sl = slice(c * BL, (c + 1) * BL)
        m = dict(shared)
        m.update(_core_idx(item[sl], x[sl]))
        in_maps.append(m)

    res = run_bass_kernel_spmd(nc, in_maps, core_ids=list(range(NC)))

    out = np.zeros((B, T), np.float32)
    for c in range(NC):
        out[c * BL:(c + 1) * BL] = unpack_pred(res.results[c]["pred"])
    return out
